# revision 6
# baseline (speedup 1.0000x reference)
"""Trainium2 Bass kernel for nn_Attention_34986803593304.

The reference module algebraically reduces to a single fused GEMM+ReLU:
  softmax over a length-1 axis is identically 1, so the attention output equals
  the V projection exactly, and the (10,B,1,20) view round-trips. Hence
    out = relu(gamma * (x @ W3.T) + x @ W4.T) = relu(x @ (gamma*W3 + W4).T)
with x [262144, 20], weights [200, 20] -> out [262144, 1, 200]. W1/W2 are dead.

Strategy: pure data parallel over batch across 8 NeuronCores (32768 rows each).
Per core, the GEMM runs on the tensor engine with the contraction dim (20) on
partitions. x is transposed/packed on the host so each 128-partition SBUF tile
holds four 32-aligned partition groups (d padded 20->32), enabling 4-way
concurrent fp32 matmuls via tile_position row groups. ReLU is applied during the
PSUM->SBUF copy (alternating scalar/vector engines), and stores are arranged so
every partition writes 12.8KB contiguous to HBM.
"""
import sys

sys.path.insert(0, "/opt/trn_rl_repo")

import numpy as np

import concourse.bass as bass
from concourse import bacc
import concourse.mybir as mybir
import concourse.tile as tile
from concourse.bass_utils import run_bass_kernel_spmd

NCORES = 8
B = 262144
BS = B // NCORES        # 32768 rows per core
D = 20                  # contraction dim
O = 200                 # output features
OP = 256                # padded N for the float32r path
P = 128
NBLK = BS // 2048       # 16 blocks of 2048 rows per core
F32 = mybir.dt.float32
F32R = mybir.dt.float32r
RELU = mybir.ActivationFunctionType.Relu

# "pack4": fp32 with 4-way tile_position packing (host-packed x, d padded to 32)
# "f32r":  float32r with N padded to 256 (plain transposed x)
MODE = "pack4"

_nc_cache = {}


def _build_nc(mode):
    nc = bacc.Bacc("TRN2", target_bir_lowering=False, debug=True)
    if mode == "pack4":
        xp = nc.declare_dram_parameter("xp", [P, NBLK * 512], F32, isOutput=False)
        wr = nc.declare_dram_parameter("wr", [P, O], F32, isOutput=False)
    else:
        xt = nc.declare_dram_parameter("xt", [D, BS], F32R, isOutput=False)
        wp = nc.declare_dram_parameter("wp", [D, OP], F32R, isOutput=False)
    out = nc.declare_dram_parameter("out", [BS, O], F32, isOutput=True)

    with tile.TileContext(nc) as tc:
        with (
            tc.tile_pool(name="singles", bufs=1) as singles,
            tc.tile_pool(name="xs", bufs=4) as xs_pool,
            tc.tile_pool(name="ob", bufs=3) as ob_pool,
            tc.tile_pool(name="ps", bufs=8, space="PSUM") as ps_pool,
        ):
            if mode == "pack4":
                # partition p=32a+d holds rows {16m+4u+a} of each block;
                # store: psum partition m at step t=4u+a is row 16m+t
                # -> out partition p covers rows 16p..16p+15: contiguous.
                out_r = out[:].rearrange("(b p t) o -> b p (t o)", p=P, t=16)
                wsb = singles.tile([P, O], F32)
                nc.sync.dma_start(out=wsb, in_=wr[:])
                xch = []
                for g in range(4):
                    xt_g = xs_pool.tile([P, 2048], F32, tag="xch")
                    nc.sync.dma_start(out=xt_g, in_=xp[:, g * 2048 : (g + 1) * 2048])
                    xch.append(xt_g)
                for blk in range(NBLK):
                    g, b = blk // 4, blk % 4
                    ob = ob_pool.tile([P, 16 * O], F32, tag="ob")
                    for u in range(4):
                        for a in range(4):
                            t = 4 * u + a
                            pt = ps_pool.tile([P, O], F32, tag="pt")
                            nc.tensor.matmul(
                                pt,
                                lhsT=xch[g][32 * a : 32 * a + D, b * 512 + u * 128 : b * 512 + (u + 1) * 128],
                                rhs=wsb[32 * a : 32 * a + D, :],
                                start=True,
                                stop=True,
                                tile_position=(32 * a, 0),
                            )
                            if a % 2 == 0:
                                nc.scalar.activation(ob[:, t * O : (t + 1) * O], pt, RELU)
                            else:
                                nc.vector.tensor_scalar_max(ob[:, t * O : (t + 1) * O], pt, 0.0)
                    nc.scalar.dma_start(out=out_r[blk], in_=ob)
            else:
                # plain layout: block blk, tile t covers rows blk*2048+t*128 ... +128
                out_r = out[:].rearrange("(b t p) o -> b p t o", p=P, t=16)
                wsb = singles.tile([D, OP], F32R)
                nc.sync.dma_start(out=wsb, in_=wp[:])
                xch = []
                for g in range(4):
                    xt_g = xs_pool.tile([D, 8192], F32R, tag="xch")
                    nc.sync.dma_start(out=xt_g, in_=xt[:, g * 8192 : (g + 1) * 8192])
                    xch.append(xt_g)
                for blk in range(NBLK):
                    g, b = blk // 4, blk % 4
                    ob = ob_pool.tile([P, 16 * O], F32, tag="ob")
                    for t in range(16):
                        pt = ps_pool.tile([P, OP], F32, tag="pt")
                        nc.tensor.matmul(
                            pt,
                            lhsT=xch[g][:, b * 2048 + t * 128 : b * 2048 + (t + 1) * 128],
                            rhs=wsb[:],
                            start=True,
                            stop=True,
                        )
                        if t % 2 == 0:
                            nc.scalar.activation(ob[:, t * O : (t + 1) * O], pt[:, :O], RELU)
                        else:
                            nc.vector.tensor_scalar_max(ob[:, t * O : (t + 1) * O], pt[:, :O], 0.0)
                    nc.scalar.dma_start(
                        out=out_r[blk], in_=ob.rearrange("p (t o) -> p t o", t=16)
                    )
    nc.compile()
    return nc


def _get_nc(mode):
    if mode not in _nc_cache:
        _nc_cache[mode] = _build_nc(mode)
    return _nc_cache[mode]


def _pack_x_pack4(x):
    """x [B, 20] -> per-core [128, 8192]; xp[c][32a+d, blk*512+u*128+m] =
    x[c*32768 + blk*2048 + 16m + 4u + a, d]."""
    y = x.reshape(NCORES, NBLK, 128, 4, 4, D)          # [c, blk, m, u, a, d]
    y = y.transpose(0, 4, 5, 1, 3, 2)                   # [c, a, d, blk, u, m]
    z = np.zeros((NCORES, 4, 32, NBLK, 4, 128), np.float32)
    z[:, :, :D] = y
    return np.ascontiguousarray(z.reshape(NCORES, P, NBLK * 512))


def run(inputs, mode=MODE, trace=False, trace_cores=None, tmpdir=None):
    x = np.asarray(inputs["x"], np.float32)
    W3 = np.asarray(inputs["W3"], np.float64)
    W4 = np.asarray(inputs["W4"], np.float64)
    gamma = float(np.asarray(inputs["gamma"]).reshape(-1)[0])
    Wc = (gamma * W3 + W4).astype(np.float32)           # [200, 20]
    wT = np.ascontiguousarray(Wc.T)                     # [20, 200]

    if mode == "pack4":
        wr = np.zeros((P, O), np.float32)
        for a in range(4):
            wr[32 * a : 32 * a + D] = wT
        xp = _pack_x_pack4(x)
        in_maps = [{"xp": xp[c], "wr": wr} for c in range(NCORES)]
    else:
        wp = np.zeros((D, OP), np.float32)
        wp[:, :O] = wT
        xt = np.ascontiguousarray(x.T.reshape(D, NCORES, BS).transpose(1, 0, 2))
        in_maps = [{"xt": xt[c]} for c in range(NCORES)]
        for m in in_maps:
            m["wp"] = wp

    nc = _get_nc(mode)
    res = run_bass_kernel_spmd(
        nc,
        in_maps,
        list(range(NCORES)),
        trace=trace,
        trace_cores=trace_cores,
        tmpdir=tmpdir,
    )
    out = np.concatenate([res.results[c]["out"] for c in range(NCORES)], axis=0)
    return out.reshape(B, 1, O), res


def kernel(**inputs):
    return run(inputs)[0]


# revision 7
# speedup vs baseline: 1.3705x; 1.3705x over previous
"""Trainium2 Bass kernel for nn_Attention_34986803593304.

The reference module algebraically reduces to a single fused GEMM+ReLU:
  softmax over a length-1 axis is identically 1, so the attention output equals
  the V projection exactly, and the (10,B,1,20) view round-trips. Hence
    out = relu(gamma * (x @ W3.T) + x @ W4.T) = relu(x @ (gamma*W3 + W4).T)
with x [262144, 20], weights [200, 20] -> out [262144, 1, 200]. W1/W2 are dead.

Strategy: pure data parallel over batch across 8 NeuronCores (32768 rows each).
Per core the GEMM runs on the tensor engine with the contraction dim on
partitions. Default mode "f16" computes x@w.T to ~fp32 accuracy with ONE fp16
matmul per 128-row tile by stacking the error-compensated split along K:
  x = xh + xl, w = wh + wl (fp16 hi/lo)  =>  x@w ~= xh@wh + xl@wh + xh@wl
  lhsT = [xh; xl; xh] (K=60), rhs = [wh; wh; wl]  (dropped xl@wl ~ 2^-22)
fp16 streams 1 col/cycle on the PE (fp32 is 4x slower and never engages the
HAM clock boost). Two 64-aligned partition groups run concurrently via
tile_position. ReLU is applied during the PSUM->SBUF copy (alternating
scalar/vector engines), and stores are arranged so every partition writes
12.8KB contiguous to HBM.
"""
import sys

sys.path.insert(0, "/opt/trn_rl_repo")

import numpy as np

import concourse.bass as bass
from concourse import bacc
import concourse.mybir as mybir
import concourse.tile as tile
from concourse.bass_utils import run_bass_kernel_spmd

NCORES = 8
B = 262144
BS = B // NCORES        # 32768 rows per core
D = 20                  # contraction dim
O = 200                 # output features
P = 128
NBLK = BS // 2048       # 16 blocks of 2048 rows per core
F32 = mybir.dt.float32
F16 = mybir.dt.float16
RELU = mybir.ActivationFunctionType.Relu

# "f16":   fp16 hi/lo K=60 single-pass (default; ~fp32 accuracy, fast PE)
# "pack4": fp32 with 4-way tile_position packing (exact fp32, slower PE)
MODE = "f16"

_nc_cache = {}


def _build_nc(mode, nblk=NBLK):
    nc = bacc.Bacc("TRN2", target_bir_lowering=False, debug=True)
    out = nc.declare_dram_parameter("out", [nblk * 2048, O], F32, isOutput=True)
    # store: psum partition m at step t holds row 16m+t of the block
    # -> out partition p covers rows 16p..16p+15: contiguous 12.8KB
    out_r = out[:].rearrange("(b p t) o -> b p (t o)", p=P, t=16)

    with tile.TileContext(nc) as tc:
        with (
            tc.tile_pool(name="singles", bufs=1) as singles,
            tc.tile_pool(name="xs", bufs=2) as xs_pool,
            tc.tile_pool(name="ob", bufs=3) as ob_pool,
            tc.tile_pool(name="ps", bufs=8, space="PSUM") as ps_pool,
        ):
            if mode == "f16":
                # partition 64q+j: j<20 xh, 20<=j<40 xl, 40<=j<60 xh (dup), pad
                # free col (blk, u, m) = blk*1024 + u*128 + m holds batch row
                # blk*2048 + 16m + 2u + q
                xp = nc.declare_dram_parameter("xp", [P, nblk * 1024], F16, isOutput=False)
                wr = nc.declare_dram_parameter("wr", [P, O], F16, isOutput=False)
                wsb = singles.tile([P, O], F16)
                nc.sync.dma_start(out=wsb, in_=wr[:])
                nchunk = max(1, nblk // 4)
                xch = {}
                for blk in range(nblk):
                    g, b = blk // 4, blk % 4
                    if b == 0:
                        cb = min(4, nblk - g * 4)  # blocks in this chunk
                        xg = xs_pool.tile([P, cb * 1024], F16, tag="xch")
                        nc.sync.dma_start(
                            out=xg, in_=xp[:, g * 4096 : g * 4096 + cb * 1024]
                        )
                        xch[g] = xg
                    ob = ob_pool.tile([P, 16 * O], F32, tag="ob")
                    for u in range(8):
                        for q in range(2):
                            t = 2 * u + q
                            pt = ps_pool.tile([P, O], F32, tag="pt")
                            nc.tensor.matmul(
                                pt,
                                lhsT=xch[g][64 * q : 64 * q + 60, b * 1024 + u * 128 : b * 1024 + (u + 1) * 128],
                                rhs=wsb[64 * q : 64 * q + 60, :],
                                start=True,
                                stop=True,
                                tile_position=(64 * q, 0),
                            )
                            if q == 0:
                                nc.scalar.activation(ob[:, t * O : (t + 1) * O], pt, RELU)
                            else:
                                nc.vector.tensor_scalar_max(ob[:, t * O : (t + 1) * O], pt, 0.0)
                    nc.scalar.dma_start(out=out_r[blk], in_=ob)
            else:
                # fp32: partition 32a+d holds rows {16m+4u+a} of each block
                xp = nc.declare_dram_parameter("xp", [P, nblk * 512], F32, isOutput=False)
                wr = nc.declare_dram_parameter("wr", [P, O], F32, isOutput=False)
                wsb = singles.tile([P, O], F32)
                nc.sync.dma_start(out=wsb, in_=wr[:])
                xch = {}
                for blk in range(nblk):
                    g, b = blk // 4, blk % 4
                    if b == 0:
                        cb = min(4, nblk - g * 4)
                        xg = xs_pool.tile([P, cb * 512], F32, tag="xch")
                        nc.sync.dma_start(
                            out=xg, in_=xp[:, g * 2048 : g * 2048 + cb * 512]
                        )
                        xch[g] = xg
                    ob = ob_pool.tile([P, 16 * O], F32, tag="ob")
                    for u in range(4):
                        for a in range(4):
                            t = 4 * u + a
                            pt = ps_pool.tile([P, O], F32, tag="pt")
                            nc.tensor.matmul(
                                pt,
                                lhsT=xch[g][32 * a : 32 * a + D, b * 512 + u * 128 : b * 512 + (u + 1) * 128],
                                rhs=wsb[32 * a : 32 * a + D, :],
                                start=True,
                                stop=True,
                                tile_position=(32 * a, 0),
                            )
                            if a % 2 == 0:
                                nc.scalar.activation(ob[:, t * O : (t + 1) * O], pt, RELU)
                            else:
                                nc.vector.tensor_scalar_max(ob[:, t * O : (t + 1) * O], pt, 0.0)
                    nc.scalar.dma_start(out=out_r[blk], in_=ob)
    nc.compile()
    return nc


def _get_nc(mode, nblk=NBLK):
    key = (mode, nblk)
    if key not in _nc_cache:
        _nc_cache[key] = _build_nc(mode, nblk)
    return _nc_cache[key]


def _pack_x_f16(x):
    """x [B, 20] f32 -> per-core [128, NBLK*1024] f16 hi/lo stacked layout."""
    xh = x.astype(np.float16)
    xl = (x - xh.astype(np.float32)).astype(np.float16)
    # [c, blk, m, u, q, d]
    sh = (NCORES, NBLK, 128, 8, 2, D)
    yh = xh.reshape(sh).transpose(0, 4, 5, 1, 3, 2)  # [c, q, d, blk, u, m]
    yl = xl.reshape(sh).transpose(0, 4, 5, 1, 3, 2)
    z = np.zeros((NCORES, 2, 64, NBLK, 8, 128), np.float16)
    z[:, :, 0:D] = yh
    z[:, :, D : 2 * D] = yl
    z[:, :, 2 * D : 3 * D] = yh
    return np.ascontiguousarray(z.reshape(NCORES, P, NBLK * 1024))


def _pack_x_f32(x):
    """x [B, 20] f32 -> per-core [128, NBLK*512] fp32 4-group layout."""
    y = x.reshape(NCORES, NBLK, 128, 4, 4, D)          # [c, blk, m, u, a, d]
    y = y.transpose(0, 4, 5, 1, 3, 2)                   # [c, a, d, blk, u, m]
    z = np.zeros((NCORES, 4, 32, NBLK, 4, 128), np.float32)
    z[:, :, :D] = y
    return np.ascontiguousarray(z.reshape(NCORES, P, NBLK * 512))


def run(inputs, mode=MODE, trace=False, trace_cores=None, tmpdir=None):
    x = np.asarray(inputs["x"], np.float32)
    W3 = np.asarray(inputs["W3"], np.float64)
    W4 = np.asarray(inputs["W4"], np.float64)
    gamma = float(np.asarray(inputs["gamma"]).reshape(-1)[0])
    Wc = (gamma * W3 + W4).astype(np.float32)           # [200, 20]
    wT = np.ascontiguousarray(Wc.T)                     # [20, 200]

    if mode == "f16":
        wh = wT.astype(np.float16)
        wl = (wT - wh.astype(np.float32)).astype(np.float16)
        wr = np.zeros((P, O), np.float16)
        for q in range(2):
            wr[64 * q : 64 * q + D] = wh
            wr[64 * q + D : 64 * q + 2 * D] = wh
            wr[64 * q + 2 * D : 64 * q + 3 * D] = wl
        xp = _pack_x_f16(x)
    else:
        wr = np.zeros((P, O), np.float32)
        for a in range(4):
            wr[32 * a : 32 * a + D] = wT
        xp = _pack_x_f32(x)
    in_maps = [{"xp": xp[c], "wr": wr} for c in range(NCORES)]

    nc = _get_nc(mode)
    res = run_bass_kernel_spmd(
        nc,
        in_maps,
        list(range(NCORES)),
        trace=trace,
        trace_cores=trace_cores,
        tmpdir=tmpdir,
    )
    out = np.concatenate([res.results[c]["out"] for c in range(NCORES)], axis=0)
    return out.reshape(B, 1, O), res


def kernel(**inputs):
    return run(inputs)[0]
